# revision 20
# baseline (speedup 1.0000x reference)
"""AgentCrossAttention Trainium2 kernel (round 2).

Sharding: data-parallel over the 512 (b,t) frames -> 64 frames per core.

Design notes:
  - bf16 into the PE array, fp32 PSUM accumulation.
  - z cast to bf16 AND transposed on the host, so contraction dim D sits on
    SBUF partitions straight off a plain, fully-coalesced DMA.
  - k and v are both token-major and share the same stationary z^T tiles
    (one ldweights feeds a v-matmul and a k-matmul).
  - rmsnorm(k): square + per-head 3D reduce on DVE -> [128,2,4,1], so the
    reciprocal+sqrt run on 8 elements per partition instead of 512 (the
    round-1 [128,512] DVE reciprocal was 3.4us/frame on the critical path).
  - Both gammas (q_gamma*k_gamma) fold into the q side as a per-partition
    scalar; k side is pure rmsnorm.
  - Softcap bounds |logit| <= 50, so exp(50*tanh) <= e^50 fits fp32 with
    room: softmax runs WITHOUT max subtraction (no reduce_max).
  - kn^T and attn^T via DMA X-bar transposes (Bacc.compile() legalizes the
    multi-wait sync that walrus's per-instruction wait slots can't hold).
  - Software pipelining: frame f+1's DMA + k/v matmuls are emitted before
    frame f's attention tail, so the PE never idles long enough for the
    HAM clock gate to re-throttle it to 1.2 GHz.
  - Wq columns / Wo rows host-permuted: per-kv-head q slices, attention
    output and final projection line up on partitions with zero shuffles.
    head h <-> (half=(h//4)%2, m=(h//8)*4+h%4), new = m*128 + half*64 + d.
  - softmax runs on full [128,256] tiles (only rows 32j+g are real; garbage
    rows are never read downstream).
"""

import numpy as np
import ml_dtypes

import concourse.bass as bass
import concourse.bacc as bacc
import concourse.mybir as mybir
import concourse.tile as tile
from concourse.bass_utils import run_bass_kernel_spmd

F32 = mybir.dt.float32
BF16 = mybir.dt.bfloat16
AF = mybir.ActivationFunctionType
AX = mybir.AxisListType

DIM = 1024
H = 16
HKV = 4
HD = 64
G = 4
B, T, S = 4, 128, 256
NCORES = 8
FPC = (B * T) // NCORES          # frames per core = 64
KT = DIM // 128                  # 8 contraction tiles
SOFT_CAP = 50.0
SCALE = HD ** -0.5


def _head_colmap():
    """new index m*128 + half*64 + d  <-  old index h*64 + d."""
    cm = np.empty(DIM, dtype=np.int64)
    for m in range(8):
        for half in range(2):
            j = (m // 4) * 2 + half
            g = m % 4
            h = 4 * j + g
            for d in range(HD):
                cm[m * 128 + half * 64 + d] = h * 64 + d
    return cm


def _build_bass():
    nc = bacc.Bacc("TRN2", target_bir_lowering=False, debug=False)
    z = nc.dram_tensor("z", [FPC, DIM, S], BF16, kind="ExternalInput")
    agT = nc.dram_tensor("agT", [128, KT, FPC], BF16, kind="ExternalInput")
    wq = nc.dram_tensor("wq", [128, KT, 8, 128], BF16, kind="ExternalInput")
    wkv = nc.dram_tensor("wkv", [128, KT, 512], BF16, kind="ExternalInput")
    wo = nc.dram_tensor("wo", [128, KT, DIM], BF16, kind="ExternalInput")
    ones = nc.dram_tensor("ones", [128, 128], BF16, kind="ExternalInput")
    g2 = nc.dram_tensor("g2", [128, 1], F32, kind="ExternalInput")
    ident = nc.dram_tensor("ident", [128, 128], BF16, kind="ExternalInput")
    rsC = nc.dram_tensor("rsC", [128, 2, 4, 1], mybir.dt.int32,
                         kind="ExternalInput")
    yT = nc.dram_tensor("yT", [128, KT, FPC], F32, kind="ExternalOutput")

    with tile.TileContext(nc) as tc:
        with (
            tc.tile_pool(name="const", bufs=1) as cpool,
            tc.tile_pool(name="persist", bufs=1) as ppool,
            tc.tile_pool(name="qsb", bufs=1) as qsb,
            tc.tile_pool(name="zt", bufs=8) as zpool,
            tc.tile_pool(name="fsb", bufs=3) as fsb,
            tc.tile_pool(name="knp", bufs=3) as knp,
            tc.tile_pool(name="atp", bufs=3) as atp,
            tc.tile_pool(name="pskv", bufs=2, space="PSUM") as pskv,
            tc.tile_pool(name="psc", bufs=1, space="PSUM") as psc,
            tc.tile_pool(name="psa", bufs=1, space="PSUM") as psa,
            tc.tile_pool(name="pso", bufs=1, space="PSUM") as pso,
        ):
            wq_sb = cpool.tile([128, KT, 8, 128], BF16)
            nc.sync.dma_start(wq_sb[:], wq[:])
            wkv_sb = cpool.tile([128, KT, 512], BF16)
            nc.sync.dma_start(wkv_sb[:], wkv[:])
            wo_sb = cpool.tile([128, KT, DIM], BF16)
            nc.sync.dma_start(wo_sb[:], wo[:])
            ones_sb = cpool.tile([128, 128], BF16)
            nc.sync.dma_start(ones_sb[:], ones[:])
            g2_sb = cpool.tile([128, 1], F32)
            nc.sync.dma_start(g2_sb[:], g2[:])
            ident_sb = cpool.tile([128, 128], BF16)
            nc.sync.dma_start(ident_sb[:], ident[:])
            rsC_sb = cpool.tile([128, 2, 4, 1], mybir.dt.int32)
            nc.sync.dma_start(rsC_sb[:], rsC[:])
            agT_sb = cpool.tile([128, KT, FPC], BF16)
            nc.sync.dma_start(agT_sb[:], agT[:])

            qgT_sb = ppool.tile([128, 8, FPC], BF16)   # normalized q^T (permuted)
            oT_sb = ppool.tile([128, KT, FPC], BF16)   # attention out^T (permuted)

            # ---------------- q projection + rmsnorm (once) ----------------
            psum_q = pskv.tile([128, 8, FPC], F32, tag="kv")
            for m in range(8):
                for kt in range(KT):
                    nc.tensor.matmul(
                        psum_q[:, m, :],
                        wq_sb[:, kt, m, :],
                        agT_sb[:, kt, :],
                        start=(kt == 0),
                        stop=(kt == KT - 1),
                    )
            qsq = qsb.tile([128, 8, FPC], BF16)
            nc.scalar.square(qsq[:], psum_q[:])
            psum_qs = pso.tile([128, 8, FPC], F32, tag="y")
            for m in range(8):
                nc.tensor.matmul(
                    psum_qs[:, m, :], ones_sb[:], qsq[:, m, :],
                    start=True, stop=True,
                )
            qinv = qsb.tile([128, 8, FPC], F32)
            nc.vector.reciprocal(qinv[:], psum_qs[:])
            qrs = qsb.tile([128, 8, FPC], F32)
            nc.scalar.activation(qrs[:], qinv[:], AF.Sqrt, scale=float(HD))
            # qgT = (q * g2[p]) * rsqrt(mean q^2): both gammas live here
            nc.vector.scalar_tensor_tensor(
                qgT_sb[:], psum_q[:], g2_sb[:], qrs[:],
                op0=mybir.AluOpType.mult, op1=mybir.AluOpType.mult,
            )

            # ---------------- per-frame attention (software-pipelined) -----
            def stage_a(f):
                """DMA + all PE projection work for frame f."""
                zT = zpool.tile([128, KT, S], BF16, tag="zT")
                nc.sync.dma_start(
                    zT[:], z[f].rearrange("(kt p) s -> p kt s", p=128))
                psum_kv = pskv.tile([128, 2, 512], F32, tag="kv")
                for st in range(2):
                    for kt in range(KT):
                        nc.tensor.matmul(
                            psum_kv[:, st, :],
                            zT[:, kt, st * 128:(st + 1) * 128],
                            wkv_sb[:, kt, :],
                            start=(kt == 0), stop=(kt == KT - 1),
                        )
                return (psum_kv,)

            def stage_b(f, psum_kv):
                """Norm + attention tail for frame f."""
                psum_v = psum_kv[:].rearrange("p st (h c) -> p st h c", h=2)[:, :, 0, :]
                psum_k = psum_kv[:].rearrange("p st (h c) -> p st h c", h=2)[:, :, 1, :]
                v_sb = fsb.tile([128, 2, 256], BF16, tag="v_sb")
                nc.scalar.copy(v_sb[:], psum_v)

                # ksq = (k/8)^2 so the summed result is mean(k^2) directly
                ksq = fsb.tile([128, 2, 256], BF16, tag="ksq")
                nc.scalar.activation(ksq[:], psum_k, AF.Square,
                                     scale=1.0 / 8.0)
                ss = fsb.tile([128, 2, 4, 1], F32, tag="ss")
                nc.vector.reduce_sum(
                    ss[:], ksq[:].rearrange("p st (j d) -> p st j d", d=HD),
                    axis=AX.X)
                # rsqrt via bit trick + 1 Newton step, all on DVE (keeps the
                # ACT LUT parked on the exp/tanh/square table set)
                sh = fsb.tile([128, 2, 4, 1], mybir.dt.int32, tag="sh")
                nc.vector.tensor_scalar(
                    sh[:], ss[:].bitcast(mybir.dt.int32), 1, None,
                    op0=mybir.AluOpType.logical_shift_right)
                r0 = fsb.tile([128, 2, 4, 1], mybir.dt.int32, tag="r0")
                nc.vector.tensor_sub(r0[:], rsC_sb[:], sh[:])
                r0f = r0[:].bitcast(F32)
                na = fsb.tile([128, 2, 4, 1], F32, tag="na")
                nc.vector.tensor_mul(na[:], ss[:], r0f)
                nb = fsb.tile([128, 2, 4, 1], F32, tag="nb")
                nc.vector.tensor_mul(nb[:], na[:], r0f)
                ncf = fsb.tile([128, 2, 4, 1], F32, tag="ncf")
                nc.vector.tensor_scalar(
                    ncf[:], nb[:], -0.5, 1.5,
                    op0=mybir.AluOpType.mult, op1=mybir.AluOpType.add)
                krs = fsb.tile([128, 2, 4, 1], F32, tag="krs")
                nc.vector.tensor_mul(krs[:], r0f, ncf[:])

                kn = fsb.tile([128, 2, 256], BF16, tag="kn")
                nc.vector.tensor_mul(
                    kn[:].rearrange("p st (j d) -> p st j d", d=HD),
                    psum_k.rearrange("p st (j d) -> p st j d", d=HD),
                    krs[:].broadcast_to((128, 2, 4, HD)))
                # kn^T via PE transposes: [p=jd%128, c=jd//128, st, s]
                psum_t = psa.tile([128, 768], BF16, tag="at")
                knT_ps = psum_t[:, 0:512].rearrange(
                    "p (c s) -> p c s", c=2)
                for st in range(2):
                    for c in range(2):
                        nc.tensor.transpose(
                            knT_ps[:, c, st * 128:(st + 1) * 128],
                            kn[:, st, c * 128:(c + 1) * 128],
                            ident_sb[:],
                        )
                knT = knp.tile([128, 2, 256], BF16, tag="knT")
                nc.scalar.copy(knT[:], knT_ps)

                # scores: per (kv-head j, s-half st): [4,128] at psum rows 32j
                comb = psc.tile([128, 264], F32, tag="sc")  # scores + outT
                for j in range(HKV):
                    h2 = 64 * (j % 2)
                    mc = (j // 2) * 4
                    nc.tensor.matmul(
                        comb[32 * j:32 * j + 4, 0:256],
                        qgT_sb[h2:h2 + 64, mc:mc + 4, f],
                        knT[h2:h2 + 64, j // 2, :],
                        start=True, stop=True,
                        tile_position=(h2, 32 * j),
                    )
                # softcap tanh + max-free softmax along s
                tcap = fsb.tile([128, S], F32, tag="tcap")
                nc.scalar.activation(
                    tcap[:], comb[:, 0:256], AF.Tanh, scale=SCALE / SOFT_CAP)
                esb = fsb.tile([128, S], F32, tag="esb")
                ssum = fsb.tile([128, 1], F32, tag="ssum")
                nc.scalar.activation(
                    esb[:], tcap[:], AF.Exp, scale=SOFT_CAP,
                    accum_out=ssum[:])
                rinv = fsb.tile([128, 1], F32, tag="rinv")
                nc.vector.reciprocal(rinv[:], ssum[:])
                attn_sb = fsb.tile([128, S], BF16, tag="attn")
                nc.vector.tensor_mul(attn_sb[:], esb[:],
                                     rinv[:].broadcast_to((128, S)))
                at_ps = psum_t[:, 512:768].rearrange(
                    "p (st s) -> p st s", st=2)
                for st in range(2):
                    nc.tensor.transpose(
                        at_ps[:, st, :],
                        attn_sb[:, st * 128:(st + 1) * 128],
                        ident_sb[:],
                    )
                attnT = atp.tile([128, 2, 128], BF16, tag="attnT")
                nc.vector.tensor_copy(attnT[:], at_ps)

                # attn @ v -> out^T [64 hd, 4 g] per j (permuted layout)
                for j in range(HKV):
                    pb = 64 * (j % 2)
                    mc = (j // 2) * 4
                    for st in range(2):
                        nc.tensor.matmul(
                            comb[pb:pb + 64, 256 + mc:256 + mc + 4],
                            v_sb[:, st, j * 64:(j + 1) * 64],
                            attnT[:, st, 32 * j:32 * j + 4],
                            start=(st == 0),
                            stop=(st == 1),
                        )
                nc.vector.tensor_copy(oT_sb[:, :, f], comb[:, 256:264])

            prev = stage_a(0)
            for f in range(FPC):
                cur = stage_a(f + 1) if f + 1 < FPC else None
                stage_b(f, *prev)
                prev = cur

            # ---------------- output projection (once) ----------------
            psum_y = pso.tile([128, KT, FPC], F32, tag="y")
            for dt in range(KT):
                for kt in range(KT):
                    nc.tensor.matmul(
                        psum_y[:, dt, :],
                        wo_sb[:, kt, dt * 128:(dt + 1) * 128],
                        oT_sb[:, kt, :],
                        start=(kt == 0),
                        stop=(kt == KT - 1),
                    )
            y_sb = qsb.tile([128, KT, FPC], F32, tag="ysb")
            nc.vector.tensor_copy(y_sb[:], psum_y[:])
            nc.gpsimd.dma_start(yT[:], y_sb[:])

    nc.compile()
    return nc


_NC_CACHE = {}
last_results = None


def kernel(agent_tokens, z_tokens, Wq, Wk, Wv, Wo, q_gamma, k_gamma):
    global last_results
    bf = ml_dtypes.bfloat16

    agent = np.ascontiguousarray(np.asarray(agent_tokens, np.float32)).reshape(
        B * T, DIM)
    zfull = np.asarray(z_tokens, np.float32).reshape(B * T, S, DIM)
    Wq = np.asarray(Wq, np.float32)
    Wk = np.asarray(Wk, np.float32)
    Wv = np.asarray(Wv, np.float32)
    Wo = np.asarray(Wo, np.float32)
    q_gamma = np.asarray(q_gamma, np.float32)
    k_gamma = np.asarray(k_gamma, np.float32)

    cm = _head_colmap()
    wq_host = np.ascontiguousarray(
        Wq[:, cm].reshape(KT, 128, 8, 128).transpose(1, 0, 2, 3)).astype(bf)
    wkv_host = np.ascontiguousarray(np.concatenate(
        [Wv.reshape(KT, 128, 256), Wk.reshape(KT, 128, 256)],
        axis=2).transpose(1, 0, 2)).astype(bf)
    wo_host = np.ascontiguousarray(
        Wo[cm, :].reshape(KT, 128, DIM).transpose(1, 0, 2)).astype(bf)
    blk = np.zeros((128, 128), np.float32)
    blk[:64, :64] = 1.0
    blk[64:, 64:] = 1.0
    ones_host = blk.astype(bf)
    g2_host = np.ascontiguousarray(
        np.tile((q_gamma * k_gamma).astype(np.float32), 2).reshape(128, 1))
    ident_host = np.eye(128, dtype=np.float32).astype(bf)
    rsC_host = np.full((128, 2, 4, 1), 0x5F3759DF, dtype=np.int32)

    if "nc" not in _NC_CACHE:
        _NC_CACHE["nc"] = _build_bass()
    nc = _NC_CACHE["nc"]

    in_maps = []
    for c in range(NCORES):
        fr = slice(c * FPC, (c + 1) * FPC)
        agT_host = np.ascontiguousarray(
            agent[fr].T.reshape(KT, 128, FPC).transpose(1, 0, 2)).astype(bf)
        # z host-transposed: [f, D, S] so D lands on SBUF partitions
        z_host = np.ascontiguousarray(
            zfull[fr].astype(bf).transpose(0, 2, 1))
        in_maps.append({
            "z": z_host, "agT": agT_host, "wq": wq_host, "wkv": wkv_host,
            "wo": wo_host, "ones": ones_host, "g2": g2_host,
            "ident": ident_host, "rsC": rsC_host,
        })

    res = run_bass_kernel_spmd(nc, in_maps, core_ids=list(range(NCORES)))
    last_results = res

    outs = []
    for c in range(NCORES):
        yT = np.asarray(res.results[c]["yT"], np.float32)   # [128, KT, FPC]
        outs.append(yT.transpose(2, 1, 0).reshape(FPC, DIM))
    return np.concatenate(outs, axis=0).reshape(B, T, DIM).astype(np.float32)


# revision 21
# speedup vs baseline: 1.1865x; 1.1865x over previous
"""AgentCrossAttention Trainium2 kernel (round 2).

Sharding: data-parallel over the 512 (b,t) frames -> 64 frames per core.

Design notes:
  - bf16 into the PE array, fp32 PSUM accumulation.
  - z cast to bf16 AND transposed on the host, so contraction dim D sits on
    SBUF partitions straight off a plain, fully-coalesced DMA.
  - k and v are both token-major and share the same stationary z^T tiles
    (one ldweights feeds a v-matmul and a k-matmul).
  - rmsnorm(k): square + per-head 3D reduce on DVE -> [128,2,4,1], so the
    reciprocal+sqrt run on 8 elements per partition instead of 512 (the
    round-1 [128,512] DVE reciprocal was 3.4us/frame on the critical path).
  - Both gammas (q_gamma*k_gamma) fold into the q side as a per-partition
    scalar; k side is pure rmsnorm.
  - Softcap bounds |logit| <= 50, so exp(50*tanh) <= e^50 fits fp32 with
    room: softmax runs WITHOUT max subtraction (no reduce_max).
  - kn^T and attn^T via DMA X-bar transposes (Bacc.compile() legalizes the
    multi-wait sync that walrus's per-instruction wait slots can't hold).
  - Software pipelining: frame f+1's DMA + k/v matmuls are emitted before
    frame f's attention tail, so the PE never idles long enough for the
    HAM clock gate to re-throttle it to 1.2 GHz.
  - Wq columns / Wo rows host-permuted: per-kv-head q slices, attention
    output and final projection line up on partitions with zero shuffles.
    head h <-> (half=(h//4)%2, m=(h//8)*4+h%4), new = m*128 + half*64 + d.
  - softmax runs on full [128,256] tiles (only rows 32j+g are real; garbage
    rows are never read downstream).
"""

import numpy as np
import ml_dtypes

import concourse.bass as bass
import concourse.bacc as bacc
import concourse.mybir as mybir
import concourse.tile as tile
from concourse.bass_utils import run_bass_kernel_spmd

F32 = mybir.dt.float32
BF16 = mybir.dt.bfloat16
AF = mybir.ActivationFunctionType
AX = mybir.AxisListType

DIM = 1024
H = 16
HKV = 4
HD = 64
G = 4
B, T, S = 4, 128, 256
NCORES = 8
FPC = (B * T) // NCORES          # frames per core = 64
KT = DIM // 128                  # 8 contraction tiles
SOFT_CAP = 50.0
SCALE = HD ** -0.5


def _head_colmap():
    """new index m*128 + half*64 + d  <-  old index h*64 + d."""
    cm = np.empty(DIM, dtype=np.int64)
    for m in range(8):
        for half in range(2):
            j = (m // 4) * 2 + half
            g = m % 4
            h = 4 * j + g
            for d in range(HD):
                cm[m * 128 + half * 64 + d] = h * 64 + d
    return cm


def _build_bass():
    nc = bacc.Bacc("TRN2", target_bir_lowering=False, debug=False)
    z = nc.dram_tensor("z", [FPC, DIM, S], BF16, kind="ExternalInput")
    agT = nc.dram_tensor("agT", [128, KT, FPC], BF16, kind="ExternalInput")
    wq = nc.dram_tensor("wq", [128, KT, 8, 128], BF16, kind="ExternalInput")
    wkv = nc.dram_tensor("wkv", [128, KT, 512], BF16, kind="ExternalInput")
    wo = nc.dram_tensor("wo", [128, KT, DIM], BF16, kind="ExternalInput")
    ones = nc.dram_tensor("ones", [128, 128], BF16, kind="ExternalInput")
    g2 = nc.dram_tensor("g2", [128, 1], F32, kind="ExternalInput")
    ident = nc.dram_tensor("ident", [128, 128], BF16, kind="ExternalInput")
    rsC = nc.dram_tensor("rsC", [128, 2, 4, 1], mybir.dt.int32,
                         kind="ExternalInput")
    yT = nc.dram_tensor("yT", [128, KT, FPC], F32, kind="ExternalOutput")

    with tile.TileContext(nc) as tc:
        with (
            tc.tile_pool(name="const", bufs=1) as cpool,
            tc.tile_pool(name="persist", bufs=1) as ppool,
            tc.tile_pool(name="qsb", bufs=1) as qsb,
            tc.tile_pool(name="zt", bufs=8) as zpool,
            tc.tile_pool(name="fsb", bufs=3) as fsb,
            tc.tile_pool(name="knp", bufs=3) as knp,
            tc.tile_pool(name="atp", bufs=3) as atp,
            tc.tile_pool(name="pskv", bufs=3, space="PSUM") as pskv,
            tc.tile_pool(name="psc", bufs=1, space="PSUM") as psc,
            tc.tile_pool(name="psa", bufs=1, space="PSUM") as psa,
        ):
            wq_sb = cpool.tile([128, KT, 8, 128], BF16)
            nc.sync.dma_start(wq_sb[:], wq[:])
            wkv_sb = cpool.tile([128, KT, 512], BF16)
            nc.sync.dma_start(wkv_sb[:], wkv[:])
            wo_sb = cpool.tile([128, KT, DIM], BF16)
            nc.sync.dma_start(wo_sb[:], wo[:])
            ones_sb = cpool.tile([128, 128], BF16)
            nc.sync.dma_start(ones_sb[:], ones[:])
            g2_sb = cpool.tile([128, 1], F32)
            nc.sync.dma_start(g2_sb[:], g2[:])
            ident_sb = cpool.tile([128, 128], BF16)
            nc.sync.dma_start(ident_sb[:], ident[:])
            rsC_sb = cpool.tile([128, 2, 4, 1], mybir.dt.int32)
            nc.sync.dma_start(rsC_sb[:], rsC[:])
            agT_sb = cpool.tile([128, KT, FPC], BF16)
            nc.sync.dma_start(agT_sb[:], agT[:])

            qgT_sb = ppool.tile([128, 8, FPC], BF16)   # normalized q^T (permuted)
            oT_sb = ppool.tile([128, KT, FPC], BF16)   # attention out^T (permuted)

            # ---------------- q projection + rmsnorm (once) ----------------
            psum_q = pskv.tile([128, 8, FPC], F32, tag="kv")
            for m in range(8):
                for kt in range(KT):
                    nc.tensor.matmul(
                        psum_q[:, m, :],
                        wq_sb[:, kt, m, :],
                        agT_sb[:, kt, :],
                        start=(kt == 0),
                        stop=(kt == KT - 1),
                    )
            qsq = qsb.tile([128, 8, FPC], BF16)
            nc.scalar.square(qsq[:], psum_q[:])
            psum_qs = pskv.tile([128, 8, FPC], F32, tag="kv")
            for m in range(8):
                nc.tensor.matmul(
                    psum_qs[:, m, :], ones_sb[:], qsq[:, m, :],
                    start=True, stop=True,
                )
            qinv = qsb.tile([128, 8, FPC], F32)
            nc.vector.reciprocal(qinv[:], psum_qs[:])
            qrs = qsb.tile([128, 8, FPC], F32)
            nc.scalar.activation(qrs[:], qinv[:], AF.Sqrt, scale=float(HD))
            # qgT = (q * g2[p]) * rsqrt(mean q^2): both gammas live here
            nc.vector.scalar_tensor_tensor(
                qgT_sb[:], psum_q[:], g2_sb[:], qrs[:],
                op0=mybir.AluOpType.mult, op1=mybir.AluOpType.mult,
            )

            # ---------------- per-frame attention (software-pipelined) -----
            def stage_a(f):
                """DMA + all PE projection work for frame f."""
                zT = zpool.tile([128, KT, S], BF16, tag="zT")
                nc.sync.dma_start(
                    zT[:], z[f].rearrange("(kt p) s -> p kt s", p=128))
                psum_kv = pskv.tile([128, 2, 512], F32, tag="kv")
                for st in range(2):
                    for kt in range(KT):
                        nc.tensor.matmul(
                            psum_kv[:, st, :],
                            zT[:, kt, st * 128:(st + 1) * 128],
                            wkv_sb[:, kt, :],
                            start=(kt == 0), stop=(kt == KT - 1),
                        )
                return (psum_kv,)

            def stage_b(f, psum_kv):
                """Norm + attention tail for frame f."""
                psum_v = psum_kv[:].rearrange("p st (h c) -> p st h c", h=2)[:, :, 0, :]
                psum_k = psum_kv[:].rearrange("p st (h c) -> p st h c", h=2)[:, :, 1, :]
                # ksq = (k/8)^2 so the summed result is mean(k^2) directly
                ksq = fsb.tile([128, 2, 256], BF16, tag="ksq")
                nc.scalar.activation(ksq[:], psum_k, AF.Square,
                                     scale=1.0 / 8.0)
                v_sb = fsb.tile([128, 2, 256], BF16, tag="v_sb")
                nc.scalar.copy(v_sb[:], psum_v)
                ss = fsb.tile([128, 2, 4, 1], F32, tag="ss")
                nc.vector.reduce_sum(
                    ss[:], ksq[:].rearrange("p st (j d) -> p st j d", d=HD),
                    axis=AX.X)
                # rsqrt via bit trick + 1 Newton step, all on DVE (keeps the
                # ACT LUT parked on the exp/tanh/square table set)
                sh = fsb.tile([128, 2, 4, 1], mybir.dt.int32, tag="sh")
                nc.vector.tensor_scalar(
                    sh[:], ss[:].bitcast(mybir.dt.int32), 1, None,
                    op0=mybir.AluOpType.logical_shift_right)
                r0 = fsb.tile([128, 2, 4, 1], mybir.dt.int32, tag="r0")
                nc.vector.tensor_sub(r0[:], rsC_sb[:], sh[:])
                r0f = r0[:].bitcast(F32)
                na = fsb.tile([128, 2, 4, 1], F32, tag="na")
                nc.vector.tensor_mul(na[:], ss[:], r0f)
                nb = fsb.tile([128, 2, 4, 1], F32, tag="nb")
                nc.vector.tensor_mul(nb[:], na[:], r0f)
                ncf = fsb.tile([128, 2, 4, 1], F32, tag="ncf")
                nc.vector.tensor_scalar(
                    ncf[:], nb[:], -0.5, 1.5,
                    op0=mybir.AluOpType.mult, op1=mybir.AluOpType.add)
                krs = fsb.tile([128, 2, 4, 1], F32, tag="krs")
                nc.vector.tensor_mul(krs[:], r0f, ncf[:])

                kn = fsb.tile([128, 2, 256], BF16, tag="kn")
                nc.vector.tensor_mul(
                    kn[:].rearrange("p st (j d) -> p st j d", d=HD),
                    psum_k.rearrange("p st (j d) -> p st j d", d=HD),
                    krs[:].broadcast_to((128, 2, 4, HD)))
                # kn^T via PE transposes: [p=jd%128, c=jd//128, st, s]
                psum_t = psa.tile([128, 768], BF16, tag="at")
                knT_ps = psum_t[:, 0:512].rearrange(
                    "p (c s) -> p c s", c=2)
                for st in range(2):
                    for c in range(2):
                        nc.tensor.transpose(
                            knT_ps[:, c, st * 128:(st + 1) * 128],
                            kn[:, st, c * 128:(c + 1) * 128],
                            ident_sb[:],
                        )
                knT = knp.tile([128, 2, 256], BF16, tag="knT")
                nc.scalar.copy(knT[:], knT_ps)

                # scores: per (kv-head j, s-half st): [4,128] at psum rows 32j
                comb = psc.tile([128, 264], F32, tag="sc")  # scores + outT
                for j in range(HKV):
                    h2 = 64 * (j % 2)
                    mc = (j // 2) * 4
                    nc.tensor.matmul(
                        comb[32 * j:32 * j + 4, 0:256],
                        qgT_sb[h2:h2 + 64, mc:mc + 4, f],
                        knT[h2:h2 + 64, j // 2, :],
                        start=True, stop=True,
                        tile_position=(h2, 32 * j),
                    )
                # softcap tanh + max-free softmax along s
                tcap = fsb.tile([128, S], F32, tag="tcap")
                nc.scalar.activation(
                    tcap[:], comb[:, 0:256], AF.Tanh, scale=SCALE / SOFT_CAP)
                esb = fsb.tile([128, S], F32, tag="esb")
                ssum = fsb.tile([128, 1], F32, tag="ssum")
                nc.scalar.activation(
                    esb[:], tcap[:], AF.Exp, scale=SOFT_CAP,
                    accum_out=ssum[:])
                rinv = fsb.tile([128, 1], F32, tag="rinv")
                nc.vector.reciprocal(rinv[:], ssum[:])
                attn_sb = fsb.tile([128, S], BF16, tag="attn")
                nc.vector.tensor_mul(attn_sb[:], esb[:],
                                     rinv[:].broadcast_to((128, S)))
                at_ps = psum_t[:, 512:768].rearrange(
                    "p (st s) -> p st s", st=2)
                for st in range(2):
                    nc.tensor.transpose(
                        at_ps[:, st, :],
                        attn_sb[:, st * 128:(st + 1) * 128],
                        ident_sb[:],
                    )
                attnT = atp.tile([128, 2, 128], BF16, tag="attnT")
                nc.vector.tensor_copy(attnT[:], at_ps)

                # attn @ v -> out^T [64 hd, 4 g] per j (permuted layout)
                for j in range(HKV):
                    pb = 64 * (j % 2)
                    mc = (j // 2) * 4
                    for st in range(2):
                        nc.tensor.matmul(
                            comb[pb:pb + 64, 256 + mc:256 + mc + 4],
                            v_sb[:, st, j * 64:(j + 1) * 64],
                            attnT[:, st, 32 * j:32 * j + 4],
                            start=(st == 0),
                            stop=(st == 1),
                        )
                nc.vector.tensor_copy(oT_sb[:, :, f], comb[:, 256:264])

            prev = stage_a(0)
            for f in range(FPC):
                cur = stage_a(f + 1) if f + 1 < FPC else None
                stage_b(f, *prev)
                prev = cur

            # ---------------- output projection (once) ----------------
            psum_y = pskv.tile([128, KT, FPC], F32, tag="kv")
            for dt in range(KT):
                for kt in range(KT):
                    nc.tensor.matmul(
                        psum_y[:, dt, :],
                        wo_sb[:, kt, dt * 128:(dt + 1) * 128],
                        oT_sb[:, kt, :],
                        start=(kt == 0),
                        stop=(kt == KT - 1),
                    )
            y_sb = qsb.tile([128, KT, FPC], F32, tag="ysb")
            nc.vector.tensor_copy(y_sb[:], psum_y[:])
            nc.gpsimd.dma_start(yT[:], y_sb[:])

    nc.compile()
    return nc


_NC_CACHE = {}
last_results = None


def kernel(agent_tokens, z_tokens, Wq, Wk, Wv, Wo, q_gamma, k_gamma):
    global last_results
    bf = ml_dtypes.bfloat16

    agent = np.ascontiguousarray(np.asarray(agent_tokens, np.float32)).reshape(
        B * T, DIM)
    zfull = np.asarray(z_tokens, np.float32).reshape(B * T, S, DIM)
    Wq = np.asarray(Wq, np.float32)
    Wk = np.asarray(Wk, np.float32)
    Wv = np.asarray(Wv, np.float32)
    Wo = np.asarray(Wo, np.float32)
    q_gamma = np.asarray(q_gamma, np.float32)
    k_gamma = np.asarray(k_gamma, np.float32)

    cm = _head_colmap()
    wq_host = np.ascontiguousarray(
        Wq[:, cm].reshape(KT, 128, 8, 128).transpose(1, 0, 2, 3)).astype(bf)
    wkv_host = np.ascontiguousarray(np.concatenate(
        [Wv.reshape(KT, 128, 256), Wk.reshape(KT, 128, 256)],
        axis=2).transpose(1, 0, 2)).astype(bf)
    wo_host = np.ascontiguousarray(
        Wo[cm, :].reshape(KT, 128, DIM).transpose(1, 0, 2)).astype(bf)
    blk = np.zeros((128, 128), np.float32)
    blk[:64, :64] = 1.0
    blk[64:, 64:] = 1.0
    ones_host = blk.astype(bf)
    g2_host = np.ascontiguousarray(
        np.tile((q_gamma * k_gamma).astype(np.float32), 2).reshape(128, 1))
    ident_host = np.eye(128, dtype=np.float32).astype(bf)
    rsC_host = np.full((128, 2, 4, 1), 0x5F3759DF, dtype=np.int32)

    if "nc" not in _NC_CACHE:
        _NC_CACHE["nc"] = _build_bass()
    nc = _NC_CACHE["nc"]

    in_maps = []
    for c in range(NCORES):
        fr = slice(c * FPC, (c + 1) * FPC)
        agT_host = np.ascontiguousarray(
            agent[fr].T.reshape(KT, 128, FPC).transpose(1, 0, 2)).astype(bf)
        # z host-transposed: [f, D, S] so D lands on SBUF partitions
        z_host = np.ascontiguousarray(
            zfull[fr].astype(bf).transpose(0, 2, 1))
        in_maps.append({
            "z": z_host, "agT": agT_host, "wq": wq_host, "wkv": wkv_host,
            "wo": wo_host, "ones": ones_host, "g2": g2_host,
            "ident": ident_host, "rsC": rsC_host,
        })

    res = run_bass_kernel_spmd(nc, in_maps, core_ids=list(range(NCORES)))
    last_results = res

    outs = []
    for c in range(NCORES):
        yT = np.asarray(res.results[c]["yT"], np.float32)   # [128, KT, FPC]
        outs.append(yT.transpose(2, 1, 0).reshape(FPC, DIM))
    return np.concatenate(outs, axis=0).reshape(B, T, DIM).astype(np.float32)
